# revision 29
# baseline (speedup 1.0000x reference)
"""Trainium2 Bass kernel for the EnhancedGNNDetector (3x GCN + GAT + pool + MLP).

Strategy (8 NeuronCores, SPMD single program):
  - Nodes sharded contiguously: core c owns dsts [c*6250, (c+1)*6250).
  - Edges (with self-loops) sorted by dst, partitioned by dst owner, then by
    (4-block group, src-half piece).  Within each (group, piece) run the edges
    are chunked into 128s regardless of block boundaries; a chunk crossing a
    block boundary issues one extra S-build + matmul.  Chunk counts per run
    are padded to the cross-core max so one static program serves all cores.
  - Aggregation: per chunk a one-hot S matrix (tensor_scalar is_equal against
    an iota row, dst-relative values as the per-partition scalar) turns the
    segment-sum into PE matmuls accumulating in PSUM.  For the GCN layers the
    dst-side 1/sqrt(deg) is folded into S via op1=mult.
  - Tables: host pre-scales x by dinv (g1), per layer the dinv-src-scaled
    features are written fp16 to a local DRAM table and AllGathered.
    src index is int16, so tables are gathered in two halves.
  - GAT: table rows are [hg fp16 interleaved (f,h) 256 | al_s f32 (8 slots) |
    pad] = 384 fp16 slots.  al_s/al_d come from extra host-packed Wg columns
    (Wg @ a_src per head).  al_d per edge is computed on device via
    S^T (PE transpose) matmul against SBUF-resident (al_d - c).  The softmax
    shift c is global per head; exp(leaky(u)-c) = max(exp(u-c),
    exp(0.2u)*exp(-0.8c)) so only Exp activations are needed.  Exp weights
    are written into message cols [256:260]; the aggregation matmul carries
    260 cols so the softmax denominator falls out of the same PSUM.
"""

import numpy as np
import concourse.bacc as bacc
import concourse.bass as bass
import concourse.mybir as mybir
import concourse.tile as tile
from concourse.bass_utils import run_bass_kernel_spmd

F16 = np.float16
N = 50000
E = 800000
NCORES = 8
NPC = N // NCORES            # 6250 nodes per core
NB = (NPC + 127) // 128      # 49 dst blocks per core
LASTB = NPC - 128 * (NB - 1)  # 106 rows in last block
HALF = 32768                 # int16 gather split
GBLK = 4                     # blocks per group
NG = (NB + GBLK - 1) // GBLK  # 13 groups (last has 1 block)
D_IN = 128
HID = 256
H3D = 128
HEADS = 4
FH = 64
GSLOT = 384                  # GAT table row slots (fp16)
SUB = 17                     # max chunks per gather sub-span
DM = HID + 4                 # GAT message cols incl exp weights
OUT = 8
NEG = 0.2

fp16 = mybir.dt.float16
fp32 = mybir.dt.float32
i16 = mybir.dt.int16
ALU = mybir.AluOpType
ACT = mybir.ActivationFunctionType


# --------------------------------------------------------------------------
# host-side schedule + per-core streams
# --------------------------------------------------------------------------

def _preprocess(x, edge_index):
    src = np.concatenate([edge_index[0], np.arange(N, dtype=np.int64)])
    dst = np.concatenate([edge_index[1], np.arange(N, dtype=np.int64)])
    deg = np.bincount(dst, minlength=N).astype(np.float32)
    dinv = np.where(deg > 0, 1.0 / np.sqrt(deg), 0.0).astype(np.float32)

    order = np.argsort(dst, kind="stable")
    s_src, s_dst = src[order], dst[order]

    core = s_dst // NPC
    blk = (s_dst % NPC) // 128
    grp = blk // GBLK
    piece = (s_src >= HALF).astype(np.int64)

    key = (core * NG + grp) * 2 + piece
    korder = np.argsort(key, kind="stable")   # stable: keeps dst order inside
    k_src, k_dst, k_key = s_src[korder], s_dst[korder], key[korder]
    bounds = np.searchsorted(k_key, np.arange(NCORES * NG * 2 + 1))
    cnt = (bounds[1:] - bounds[:-1]).reshape(NCORES, NG, 2)
    run_ch = (-(-cnt // 128)).max(axis=0)     # [NG, 2] cross-core chunk counts

    # canonical chunk order: for g, for piece, run chunks
    spans = []           # (start_chunk, n_chunks, piece, group)
    pos = 0
    for g in range(NG):
        for p in range(2):
            n = int(run_ch[g, p])
            if n:
                spans.append((pos, n, p, g))
                pos += n
    NCH = pos

    # per-core streams + cross-core chunk block range
    def wrap(stream):
        return np.ascontiguousarray(np.tile(stream.reshape(-1, 16).T.copy(), (8, 1)))

    blo = np.full(NCH, 1 << 30, np.int64)
    bhi = np.full(NCH, -1, np.int64)
    idxs_all, dstrel_all, dinvd_all = [], [], []
    for c in range(NCORES):
        idx_stream = np.zeros(NCH * 128, np.int16)
        rel_stream = np.full(NCH * 128, -1.0, np.float32)
        dvd_stream = np.zeros(NCH * 128, np.float32)
        for (start, n_ch, p, g) in spans:
            k = (c * NG + g) * 2 + p
            e0, e1 = bounds[k], bounds[k + 1]
            n = e1 - e0
            cap = n_ch * 128
            assert n <= cap
            es, ed = k_src[e0:e1], k_dst[e0:e1]
            q = start * 128
            idx_stream[q:q + n] = (es - (HALF if p else 0)).astype(np.int16)
            rel = ed - c * NPC - g * GBLK * 128       # rel to group base
            rel_stream[q:q + n] = rel.astype(np.float32)
            dvd_stream[q:q + n] = dinv[ed]
            # chunk block range (within group), cross-core union
            eblk = rel // 128
            for kk in range(n_ch):
                s0, s1 = kk * 128, min(n, kk * 128 + 128)
                if s0 >= s1:
                    continue
                b0 = g * GBLK + int(eblk[s0])
                b1 = g * GBLK + int(eblk[s1 - 1])
                gi = start + kk
                blo[gi] = min(blo[gi], b0)
                bhi[gi] = max(bhi[gi], b1)
        idxs_all.append(wrap(idx_stream))
        dstrel_all.append(rel_stream)
        dinvd_all.append(dvd_stream)

    # entries: (chunk, block, iota_off) per chunk, static across cores
    entries = []         # per chunk: list of (block, off)
    for (start, n_ch, p, g) in spans:
        for kk in range(n_ch):
            gi = start + kk
            ent = []
            if bhi[gi] >= 0:
                for b in range(int(blo[gi]), int(bhi[gi]) + 1):
                    ent.append((b, b - g * GBLK))
            else:
                ent.append((g * GBLK, 0))   # fully-pad chunk: harmless target
            entries.append(ent)
    total_mm = np.zeros(NB, np.int64)
    for ent in entries:
        for (b, off) in ent:
            total_mm[b] += 1

    dinv_blocks = []
    for c in range(NCORES):
        dv = np.ones(NB * 128, np.float32)
        dv[:NPC] = dinv[c * NPC:(c + 1) * NPC]
        dinv_blocks.append(np.ascontiguousarray(dv.reshape(NB, 128).T))  # [128, NB]

    groups = []
    for g in range(NG):
        gspans = [s for s in spans if s[3] == g]
        blocks = list(range(g * GBLK, min((g + 1) * GBLK, NB)))
        groups.append((blocks, gspans))

    return {
        "NCH": NCH, "spans": spans, "groups": groups, "entries": entries,
        "total_mm": total_mm,
        "idxs": idxs_all, "dstrel": dstrel_all, "dinvd": dinvd_all,
        "dinv": dinv_blocks, "dinv_full": dinv,
    }


# --------------------------------------------------------------------------
# device program
# --------------------------------------------------------------------------

def _build(sched, repeat=1, no_cc=False):
    NCH = sched["NCH"]
    groups = sched["groups"]
    entries = sched["entries"]
    total_mm = sched["total_mm"]

    nc = bacc.Bacc("TRN2", target_bir_lowering=False, debug=False,
                   num_devices=NCORES, num_swdge_queues=4)

    # ---------------- external tensors ----------------
    g1in = nc.dram_tensor("g1in", [NPC, D_IN], fp16, kind="ExternalInput")
    idxs_d = nc.dram_tensor("idxs_d", [128, NCH * 8], i16, kind="ExternalInput")
    dstrel_d = nc.dram_tensor("dstrel_d", [128, NCH], fp32, kind="ExternalInput")
    dinvd_d = nc.dram_tensor("dinvd_d", [128, NCH], fp32, kind="ExternalInput")
    dinv_d = nc.dram_tensor("dinv_d", [128, NB], fp32, kind="ExternalInput")
    w1_d = nc.dram_tensor("w1_d", [128, HID], fp16, kind="ExternalInput")
    w2_d = nc.dram_tensor("w2_d", [128, 2 * HID], fp16, kind="ExternalInput")
    w3_d = nc.dram_tensor("w3_d", [128, 2 * H3D], fp16, kind="ExternalInput")
    wg_d = nc.dram_tensor("wg_d", [128, HID + 8], fp16, kind="ExternalInput")
    b1_d = nc.dram_tensor("b1_d", [1, HID], fp16, kind="ExternalInput")
    b2_d = nc.dram_tensor("b2_d", [1, HID], fp16, kind="ExternalInput")
    b3_d = nc.dram_tensor("b3_d", [1, H3D], fp16, kind="ExternalInput")
    bg_d = nc.dram_tensor("bg_d", [128, HID], fp16, kind="ExternalInput")
    wc1_d = nc.dram_tensor("wc1_d", [128, 2 * 128], fp32, kind="ExternalInput")
    wc2_d = nc.dram_tensor("wc2_d", [128, 64], fp32, kind="ExternalInput")
    wc3_d = nc.dram_tensor("wc3_d", [64, 8], fp32, kind="ExternalInput")
    bc1_d = nc.dram_tensor("bc1_d", [128, 1], fp32, kind="ExternalInput")
    bc2_d = nc.dram_tensor("bc2_d", [64, 1], fp32, kind="ExternalInput")
    bc3_d = nc.dram_tensor("bc3_d", [8, 1], fp32, kind="ExternalInput")
    rowmask_d = nc.dram_tensor("rowmask_d", [128, 1], fp32, kind="ExternalInput")
    out_d = nc.dram_tensor("out_d", [8, 1], fp32, kind="ExternalOutput")

    # internal DRAM tables
    g1loc = nc.dram_tensor("g1loc", [NPC, D_IN], fp16)
    g1full = nc.dram_tensor("g1full", [N, D_IN], fp16, addr_space="Shared")
    g2loc = nc.dram_tensor("g2loc", [NPC, HID], fp16)
    g2full = nc.dram_tensor("g2full", [N, HID], fp16, addr_space="Shared")
    g3loc = nc.dram_tensor("g3loc", [NPC, H3D], fp16)
    g3full = nc.dram_tensor("g3full", [N, H3D], fp16, addr_space="Shared")
    gtloc = nc.dram_tensor("gtloc", [NPC, GSLOT], fp16)
    gtfull = nc.dram_tensor("gtfull", [N, GSLOT], fp16, addr_space="Shared")
    arin = nc.dram_tensor("arin", [128, 2], fp32)
    arout = nc.dram_tensor("arout", [128, 2], fp32, addr_space="Shared")

    RG = [list(range(NCORES))]

    with tile.TileContext(nc) as tc:
        import contextlib
        es = contextlib.ExitStack()
        with es:
            pers = es.enter_context(tc.tile_pool(name="pers", bufs=1))
            # ---------- persistent SBUF ----------
            idxs = pers.tile([128, NCH * 8], i16)
            nc.sync.dma_start(idxs[:], idxs_d[:])
            dstrel = pers.tile([128, NCH], fp32)
            nc.sync.dma_start(dstrel[:], dstrel_d[:])
            dinvd = pers.tile([128, NCH], fp32)
            nc.sync.dma_start(dinvd[:], dinvd_d[:])
            dinv = pers.tile([128, NB], fp32)
            nc.sync.dma_start(dinv[:], dinv_d[:])

            w1 = pers.tile([128, HID], fp16); nc.sync.dma_start(w1[:], w1_d[:])
            w2 = pers.tile([128, 2 * HID], fp16); nc.sync.dma_start(w2[:], w2_d[:])
            w3 = pers.tile([128, 2 * H3D], fp16); nc.sync.dma_start(w3[:], w3_d[:])
            wg = pers.tile([128, HID + 8], fp16); nc.sync.dma_start(wg[:], wg_d[:])
            b1r = pers.tile([1, HID], fp16); nc.sync.dma_start(b1r[:], b1_d[:])
            b2r = pers.tile([1, HID], fp16); nc.sync.dma_start(b2r[:], b2_d[:])
            b3r = pers.tile([1, H3D], fp16); nc.sync.dma_start(b3r[:], b3_d[:])
            bgr = pers.tile([128, HID], fp16); nc.sync.dma_start(bgr[:], bg_d[:])
            wc1 = pers.tile([128, 2 * 128], fp32); nc.sync.dma_start(wc1[:], wc1_d[:])
            wc2 = pers.tile([128, 64], fp32); nc.sync.dma_start(wc2[:], wc2_d[:])
            wc3 = pers.tile([64, 8], fp32); nc.sync.dma_start(wc3[:], wc3_d[:])
            bc1 = pers.tile([128, 1], fp32); nc.sync.dma_start(bc1[:], bc1_d[:])
            bc2 = pers.tile([64, 1], fp32); nc.sync.dma_start(bc2[:], bc2_d[:])
            bc3 = pers.tile([8, 1], fp32); nc.sync.dma_start(bc3[:], bc3_d[:])
            rowmask = pers.tile([128, 1], fp32); nc.sync.dma_start(rowmask[:], rowmask_d[:])

            # iota rows for S builds (values off*128 .. off*128+127), offs 0..3
            iotas = []
            for off in range(GBLK):
                it_i = pers.tile([128, 128], i16, name=f"it_i{off}")
                nc.gpsimd.iota(it_i[:], pattern=[[1, 128]], base=off * 128,
                               channel_multiplier=0)
                it_f = pers.tile([128, 128], fp16, name=f"it_f{off}")
                nc.vector.tensor_copy(it_f[:], it_i[:])
                iotas.append(it_f)
            iop_i = pers.tile([128, 1], i16)
            nc.gpsimd.iota(iop_i[:], pattern=[[1, 1]], base=0, channel_multiplier=1)
            iop_f = pers.tile([128, 1], fp16)
            nc.vector.tensor_copy(iop_f[:], iop_i[:])
            ident = pers.tile([128, 128], fp16)
            nc.vector.tensor_tensor(
                ident[:], iop_f[:].broadcast_to([128, 128]), iotas[0][:],
                op=ALU.is_equal)
            ones_r = pers.tile([1, 128], fp16)
            nc.vector.memset(ones_r[:], 1.0)
            ones_c = pers.tile([128, 1], fp16)
            nc.vector.memset(ones_c[:], 1.0)

            als_all = pers.tile([128, NB, HEADS], fp32)
            ald_all = pers.tile([128, NB, HEADS], fp32)
            ald_c = pers.tile([128, NB, HEADS], fp16)
            crep = pers.tile([128, HEADS], fp32)
            krep = pers.tile([128, HEADS], fp16)

            def rows(b):
                return LASTB if b == NB - 1 else 128

            # ---------- helpers ----------
            def transpose_to_sbuf(pool, psum_pool, src16, nslab, tag):
                out = pool.tile([128, nslab, 128], fp16, tag=tag, name=f"tT_{tag}")
                pt = psum_pool.tile([128, nslab, 128], fp16, tag="tr", name="pt_tr", bufs=1)
                for s in range(nslab):
                    nc.tensor.transpose(pt[:, s, :], src16[:, s * 128:(s + 1) * 128], ident[:])
                nc.scalar.copy(out[:], pt[:])
                return out

            qctr = [0]

            def next_q():
                qctr[0] += 1
                return qctr[0] % 4

            def gather_into(m_tile, table, start_chunk, n_chunks, elem):
                nc.gpsimd.dma_gather(
                    m_tile[:, 0:n_chunks, :], table,
                    idxs[:, start_chunk * 8:(start_chunk + n_chunks) * 8],
                    num_idxs=n_chunks * 128, num_idxs_reg=n_chunks * 128,
                    elem_size=elem, single_packet=False, queue_num=next_q())

            def maybe_cc(kind, op, replica_groups, ins, outs):
                if no_cc:
                    nrow = ins[0].shape[0]
                    h = nrow // 2
                    nc.sync.dma_start(outs[0].tensor[0:h], ins[0][0:h])
                    nc.sync.dma_start(outs[0].tensor[h:nrow], ins[0][h:nrow])
                else:
                    nc.gpsimd.collective_compute(kind, op, replica_groups=replica_groups,
                                                 ins=ins, outs=outs)

            def build_S(pool, start, n_ch, tag, weighted):
                """One S tile per span; one is_equal per (chunk, block-entry)."""
                nent = sum(len(entries[start + kk]) for kk in range(n_ch))
                S = pool.tile([128, nent, 128], fp16, tag=tag, name=f"S_{tag}", bufs=4)
                ei = 0
                emap = []    # (chunk kk, block, entry index)
                for kk in range(n_ch):
                    for (b, off) in entries[start + kk]:
                        if weighted:
                            nc.vector.tensor_scalar(
                                S[:, ei, :], iotas[off][:],
                                dstrel[:, start + kk:start + kk + 1],
                                dinvd[:, start + kk:start + kk + 1],
                                op0=ALU.is_equal, op1=ALU.mult)
                        else:
                            nc.vector.tensor_scalar(
                                S[:, ei, :], iotas[off][:],
                                dstrel[:, start + kk:start + kk + 1], None,
                                op0=ALU.is_equal)
                        emap.append((kk, b, ei))
                        ei += 1
                return S, emap

            def run_body(rep):
                nc.sync.dma_start(g1loc[:], g1in[:])
                maybe_cc("AllGather", ALU.bypass, RG, [g1loc[:]], [g1full[:]])

                # ================= GCN layer runner =================
                h1_pool = tc.tile_pool(name=f"h1pool_{rep}", bufs=1)
                h1_ctx = h1_pool.__enter__()
                h1_all = h1_ctx.tile([128, NB, HID], fp16)

                def gcn_layer(lname, table_full, D, evict_fn, stop_in_evict=False,
                              weighted_S=False):
                    with (tc.tile_pool(name=f"{lname}_sb_{rep}", bufs=2) as lp,
                          tc.tile_pool(name=f"{lname}_ps_{rep}", bufs=5, space="PSUM") as pp,
                          tc.tile_pool(name=f"{lname}_wps_{rep}", bufs=2, space="PSUM") as wp):
                        tab_lo = table_full[0:HALF, :]
                        tab_hi = table_full[HALF:N, :]
                        for (blocks, gspans) in groups:
                            paggs = {}
                            first = {}
                            done = {b: 0 for b in blocks}
                            for b in blocks:
                                paggs[b] = pp.tile([128, D], fp32, tag="agg", name="pagg")
                                first[b] = True
                            for (start0, n_ch0, p, g) in gspans:
                                tab = tab_hi if p else tab_lo
                                for s0 in range(0, n_ch0, SUB):
                                    start = start0 + s0
                                    n_ch = min(SUB, n_ch0 - s0)
                                    m = lp.tile([128, n_ch, D], fp16, tag="m", name="m", bufs=6)
                                    gather_into(m, tab, start, n_ch, D)
                                    S, emap = build_S(lp, start, n_ch, "s", weighted=weighted_S)
                                    for (kk, b, ei) in emap:
                                        done[b] += 1
                                        stop = (done[b] == total_mm[b]) and not stop_in_evict
                                        nc.tensor.matmul(
                                            paggs[b][:], S[:, ei, :], m[:, kk, :],
                                            start=first[b], stop=stop)
                                        first[b] = False
                            for b in blocks:
                                evict_fn(b, paggs[b], lp, wp)

                # ---------- layer 1 ----------
                def evict1(b, pagg, lp, wp):
                    r = rows(b)
                    a1s = lp.tile([128, D_IN], fp16, tag="ev1", name="a1s")
                    nc.scalar.activation(a1s[:], pagg[:], ACT.Copy, scale=dinv[:, b:b + 1])
                    a1T = transpose_to_sbuf(lp, wp, a1s, 1, "ev1T")
                    ph = wp.tile([128, HID], fp32, tag="wout", name="ph1")
                    nc.tensor.matmul(ph[:], a1T[:, 0, :], w1[:], start=True, stop=False)
                    nc.tensor.matmul(ph[:], ones_r[:], b1r[:], start=False, stop=True)
                    h1t = h1_all[:, b, :]
                    nc.scalar.activation(h1t, ph[:], ACT.Relu)
                    g2t = lp.tile([128, HID], fp16, tag="ev1g", name="g2t")
                    nc.scalar.activation(g2t[:], h1t, ACT.Copy, scale=dinv[:, b:b + 1])
                    nc.sync.dma_start(g2loc[b * 128:b * 128 + r, :], g2t[:r, :])

                gcn_layer("L1", g1full, D_IN, evict1)
                maybe_cc("AllGather", ALU.bypass, RG, [g2loc[:]], [g2full[:]])

                # ---------- layer 2 (+ residual + L3 transform) ----------
                def evict2(b, pagg, lp, wp):
                    r = rows(b)
                    a2s = lp.tile([128, HID], fp16, tag="ev2", name="a2s")
                    nc.scalar.activation(a2s[:], pagg[:], ACT.Copy, scale=dinv[:, b:b + 1])
                    a2T = transpose_to_sbuf(lp, wp, a2s, 2, "ev2T")
                    ph = wp.tile([128, HID], fp32, tag="wout", name="ph2")
                    nc.tensor.matmul(ph[:], a2T[:, 0, :], w2[:, 0:HID], start=True, stop=False)
                    nc.tensor.matmul(ph[:], a2T[:, 1, :], w2[:, HID:2 * HID], start=False, stop=False)
                    nc.tensor.matmul(ph[:], ones_r[:], b2r[:], start=False, stop=True)
                    r2 = lp.tile([128, HID], fp16, tag="ev2r", name="r2")
                    nc.scalar.activation(r2[:], ph[:], ACT.Relu)
                    h2t16 = lp.tile([128, HID], fp16, tag="ev2h6", name="h2t16")
                    nc.vector.tensor_tensor(h2t16[:], r2[:], h1_all[:, b, :], op=ALU.add)
                    h2T = transpose_to_sbuf(lp, wp, h2t16, 2, "ev2hT")
                    pt3 = wp.tile([128, H3D], fp32, tag="wout", name="pt3")
                    nc.tensor.matmul(pt3[:], h2T[:, 0, :], w3[:, 0:H3D], start=True, stop=False)
                    nc.tensor.matmul(pt3[:], h2T[:, 1, :], w3[:, H3D:2 * H3D], start=False, stop=True)
                    g3t = lp.tile([128, H3D], fp16, tag="ev2g", name="g3t")
                    nc.scalar.activation(g3t[:], pt3[:], ACT.Copy, scale=dinv[:, b:b + 1])
                    nc.sync.dma_start(g3loc[b * 128:b * 128 + r, :], g3t[:r, :])

                gcn_layer("L2", g2full, HID, evict2)
                h1_pool.__exit__(None, None, None)
                maybe_cc("AllGather", ALU.bypass, RG, [g3loc[:]], [g3full[:]])

                # ---------- layer 3 aggregation + GAT prep ----------
                def evict3(b, pagg, lp, wp):
                    r = rows(b)
                    # bias add closes the aggregation psum
                    nc.tensor.matmul(pagg[:], ones_r[:], b3r[:], start=False, stop=True)
                    h3t16 = lp.tile([128, H3D], fp16, tag="ev3h", name="h3t16")
                    nc.scalar.activation(h3t16[:], pagg[:], ACT.Relu)
                    h3T = transpose_to_sbuf(lp, wp, h3t16, 1, "ev3T")
                    phg = wp.tile([128, HID + 8], fp32, tag="wout", name="phg")
                    nc.tensor.matmul(phg[:], h3T[:, 0, :], wg[:], start=True, stop=True)
                    nc.scalar.copy(als_all[:, b, :], phg[:, HID:HID + 4])
                    nc.scalar.copy(ald_all[:, b, :], phg[:, HID + 4:HID + 8])
                    tabt = lp.tile([128, GSLOT], fp16, tag="ev3tab", name="tabt")
                    nc.scalar.copy(tabt[:, 0:HID], phg[:, 0:HID])
                    nc.scalar.copy(tabt[:, HID:HID + 4], phg[:, HID:HID + 4])
                    nc.sync.dma_start(gtloc[b * 128:b * 128 + r, :], tabt[:r, :])

                gcn_layer("L3", g3full, H3D, evict3, stop_in_evict=True, weighted_S=True)

                # shift constants c[h] = max(ub, 0.2*ub), ub = max al_s + max al_d
                cps = contextlib.ExitStack()
                cp = cps.enter_context(tc.tile_pool(name=f"cp_{rep}", bufs=1))
                cpp = cps.enter_context(tc.tile_pool(name=f"cpp_{rep}", bufs=1, space="PSUM"))
                m1 = cp.tile([128, HEADS], fp32)
                nc.vector.tensor_reduce(
                    m1[:], als_all[:].rearrange("p b h -> p h b"),
                    axis=mybir.AxisListType.X, op=ALU.max)
                m2 = cp.tile([128, HEADS], fp32)
                nc.vector.tensor_reduce(
                    m2[:], ald_all[:].rearrange("p b h -> p h b"),
                    axis=mybir.AxisListType.X, op=ALU.max)
                m1_16 = cp.tile([128, HEADS], fp16)
                nc.vector.tensor_copy(m1_16[:], m1[:])
                m2_16 = cp.tile([128, HEADS], fp16)
                nc.vector.tensor_copy(m2_16[:], m2[:])
                pmt1 = cpp.tile([HEADS, 128], fp16, tag="pmt1", name="pmt1")
                nc.tensor.transpose(pmt1[:], m1_16[:], ident[:])
                pmt2 = cpp.tile([HEADS, 128], fp16, tag="pmt2", name="pmt2")
                nc.tensor.transpose(pmt2[:], m2_16[:], ident[:])
                mt = cp.tile([HEADS, 2 * 128], fp32)
                nc.scalar.copy(mt[:, 0:128], pmt1[:])
                nc.scalar.copy(mt[:, 128:256], pmt2[:])
                ms = cp.tile([HEADS, 2], fp32)
                nc.vector.tensor_reduce(
                    ms[:], mt[:].rearrange("p (a j) -> p a j", a=2),
                    axis=mybir.AxisListType.X, op=ALU.max)
                ub = cp.tile([HEADS, 1], fp32)
                nc.vector.tensor_tensor(ub[:], ms[:, 0:1], ms[:, 1:2], op=ALU.add)
                ub2 = cp.tile([HEADS, 1], fp32)
                nc.vector.tensor_scalar(ub2[:], ub[:], 0.2, None, op0=ALU.mult)
                cc = cp.tile([HEADS, 1], fp32)
                nc.vector.tensor_tensor(cc[:], ub[:], ub2[:], op=ALU.max)
                cc16 = cp.tile([HEADS, 1], fp16)
                nc.vector.tensor_copy(cc16[:], cc[:])
                pcr = cpp.tile([1, HEADS], fp16)
                nc.tensor.transpose(pcr[:], cc16[:HEADS, :], ident[0:HEADS, 0:HEADS])
                pcr_sb = cp.tile([1, HEADS], fp16)
                nc.scalar.copy(pcr_sb[:], pcr[:])
                pcrep = cpp.tile([128, HEADS], fp32)
                nc.tensor.matmul(pcrep[:], ones_r[:], pcr_sb[:], start=True, stop=True)
                nc.scalar.copy(crep[:], pcrep[:])
                # krep = exp(-0.8 c); ald_c = al_d - c (fp16)
                nc.scalar.activation(krep[:], crep[:], ACT.Exp, scale=-0.8)
                nc.vector.tensor_tensor(
                    ald_c[:], ald_all[:],
                    crep[:].unsqueeze(1).broadcast_to([128, NB, HEADS]),
                    op=ALU.subtract)
                cps.close()

                maybe_cc("AllGather", ALU.bypass, RG, [gtloc[:]], [gtfull[:]])

                # ================= GAT layer =================
                plp_cm = tc.tile_pool(name=f"pool_ps_{rep}", bufs=1, space="PSUM")
                plp = plp_cm.__enter__()
                ppool = plp.tile([128, 2], fp32, tag="pp", name="ppool")
                ppool0 = ppool[:, 0:1]
                ppool1 = ppool[:, 1:2]
                with (tc.tile_pool(name=f"gat_sb_{rep}", bufs=2) as gp,
                      tc.tile_pool(name=f"gat_ps_{rep}", bufs=4, space="PSUM") as gpp,
                      tc.tile_pool(name=f"gat_tps_{rep}", bufs=2, space="PSUM") as tpp):
                    tab_lo = gtfull[0:HALF, :]
                    tab_hi = gtfull[HALF:N, :]
                    first = {b: True for b in range(NB)}
                    done = {b: 0 for b in range(NB)}
                    paggs = {}

                    def gat_span(start, n_ch, tab, pgsb, densb, bidx):
                        m = gp.tile([128, n_ch, GSLOT], fp16, tag="gm", name="gm", bufs=5)
                        gather_into(m, tab, start, n_ch, GSLOT)
                        S, emap = build_S(gp, start, n_ch, "gs", weighted=False)
                        nent = len(emap)
                        # S^T for al_d-per-edge, batched 8 per psum bank
                        ST = gp.tile([128, nent, 128], fp16, tag="gst", name="gst", bufs=4)
                        for i0 in range(0, nent, 8):
                            nb8 = min(8, nent - i0)
                            pst = tpp.tile([128, 8, 128], fp16, tag="st", name="pst", bufs=2)
                            for j in range(nb8):
                                nc.tensor.transpose(pst[:, j, :], S[:, i0 + j, :], ident[:])
                            nc.scalar.copy(ST[:, i0:i0 + nb8, :], pst[:, 0:nb8, :])
                        # u = al_s + (al_d - c) via S^T matmul
                        ups = tpp.tile([128, n_ch, HEADS], fp32, tag="u", name="ups", bufs=1)
                        for kk in range(n_ch):
                            nc.tensor.matmul(
                                ups[:, kk, :], ident[:], m[:, kk, HID:HID + 4],
                                start=True, stop=False)
                        for (kk, b, ei) in emap:
                            last = (ei == nent - 1) or emap[ei + 1][0] != kk
                            nc.tensor.matmul(
                                ups[:, kk, :], ST[:, ei, :], ald_c[:, b, :],
                                start=False, stop=last)
                        e1 = gp.tile([128, n_ch, HEADS], fp16, tag="ge1", name="ge1", bufs=3)
                        nc.scalar.activation(
                            e1[:].rearrange("p a b -> p (a b)"),
                            ups[:].rearrange("p a b -> p (a b)"), ACT.Exp)
                        e2 = gp.tile([128, n_ch, HEADS], fp16, tag="ge2", name="ge2", bufs=3)
                        nc.scalar.activation(
                            e2[:].rearrange("p a b -> p (a b)"),
                            ups[:].rearrange("p a b -> p (a b)"), ACT.Exp, scale=NEG)
                        nc.vector.tensor_tensor(
                            e2[:], e2[:],
                            krep[:].unsqueeze(1).broadcast_to([128, n_ch, HEADS]),
                            op=ALU.mult)
                        nc.vector.tensor_tensor(
                            m[:, :, HID:HID + 4], e1[:], e2[:], op=ALU.max)
                        nc.vector.tensor_tensor(
                            m[:, :, 0:HID].rearrange("p k (f h) -> p k f h", h=HEADS),
                            m[:, :, 0:HID].rearrange("p k (f h) -> p k f h", h=HEADS),
                            m[:, :, HID:HID + 4].unsqueeze(2).broadcast_to(
                                [128, n_ch, FH, HEADS]),
                            op=ALU.mult)
                        for (kk, b, ei) in emap:
                            done[b] += 1
                            stop = (done[b] == total_mm[b])
                            nc.tensor.matmul(
                                paggs[b][:], S[:, ei, :], m[:, kk, 0:DM],
                                start=first[b], stop=stop)
                            first[b] = False
                            if stop:
                                i = bidx[b]
                                pg = paggs.pop(b)
                                nc.scalar.copy(pgsb[:, i, :], pg[:, 0:HID])
                                nc.scalar.copy(densb[:, i, :], pg[:, HID:HID + 4])

                    def gat_evict_group(blocks, pgsb, densb):
                        nb = len(blocks)
                        nc.vector.tensor_scalar(
                            densb[:, 0:nb, :], densb[:, 0:nb, :], 1e-30, None,
                            op0=ALU.max)
                        rden = gp.tile([128, GBLK, HEADS], fp32, tag="grden", name="grden")
                        nc.vector.reciprocal(rden[:, 0:nb, :], densb[:, 0:nb, :])
                        t2 = gp.tile([128, GBLK, HID], fp16, tag="gt2", name="gt2")
                        nc.vector.tensor_tensor(
                            t2[:, 0:nb, :].rearrange("p b (f h) -> p b f h", h=HEADS),
                            pgsb[:, 0:nb, :].rearrange("p b (f h) -> p b f h", h=HEADS),
                            rden[:, 0:nb, :].unsqueeze(2).broadcast_to(
                                [128, nb, FH, HEADS]),
                            op=ALU.mult)
                        nc.vector.tensor_tensor(
                            t2[:, 0:nb, :], t2[:, 0:nb, :],
                            bgr[:].unsqueeze(1).broadcast_to([128, nb, HID]),
                            op=ALU.add)
                        hatt = gp.tile([128, GBLK, HID], fp16, tag="ghat", name="ghat")
                        nc.scalar.activation(
                            hatt[:, 0:nb, :].rearrange("p b d -> p (b d)"),
                            t2[:, 0:nb, :].rearrange("p b d -> p (b d)"), ACT.Relu)
                        for i, b in enumerate(blocks):
                            if rows(b) < 128:
                                nc.vector.tensor_scalar(
                                    hatt[:, i, :], hatt[:, i, :], rowmask[:], None,
                                    op0=ALU.mult)
                            nc.tensor.matmul(ppool0, hatt[:, i, 0:128], ones_c[:],
                                             start=(b == 0), stop=(b == NB - 1))
                            nc.tensor.matmul(ppool1, hatt[:, i, 128:256], ones_c[:],
                                             start=(b == 0), stop=(b == NB - 1))

                    for (blocks, gspans) in groups:
                        for b in blocks:
                            paggs[b] = gpp.tile([128, DM], fp32, tag="gagg", name="gagg")
                        pgsb = gp.tile([128, GBLK, HID], fp16, tag="gpg", name="gpg")
                        densb = gp.tile([128, GBLK, HEADS], fp32, tag="gden", name="gden")
                        bidx = {b: i for i, b in enumerate(blocks)}
                        for (start0, n_ch0, p, g) in gspans:
                            for s0 in range(0, n_ch0, SUB):
                                gat_span(start0 + s0, min(SUB, n_ch0 - s0),
                                         tab_hi if p else tab_lo, pgsb, densb, bidx)
                        gat_evict_group(blocks, pgsb, densb)

                # ---------- pooling + AllReduce + MLP ----------
                with (tc.tile_pool(name=f"mlp_sb_{rep}", bufs=1) as mp,
                      tc.tile_pool(name=f"mlp_ps_{rep}", bufs=1, space="PSUM") as mpp):
                    pool_sb = mp.tile([128, 2], fp32, name="pool_sb")
                    nc.scalar.copy(pool_sb[:], ppool[:])
                    nc.sync.dma_start(arin[:], pool_sb[:])
                    maybe_cc("AllReduce", ALU.add, RG, [arin[:]], [arout[:]])
                    pooled = mp.tile([128, 2], fp32, name="pooled")
                    nc.sync.dma_start(pooled[:], arout[:])
                    nc.vector.tensor_scalar(pooled[:], pooled[:], 1.0 / N, None, op0=ALU.mult)
                    pz1 = mpp.tile([128, 1], fp32, tag="pz", name="pz1")
                    nc.tensor.matmul(pz1[:], wc1[:, 0:128], pooled[:, 0:1], start=True, stop=False)
                    nc.tensor.matmul(pz1[:], wc1[:, 128:256], pooled[:, 1:2], start=False, stop=True)
                    z1 = mp.tile([128, 1], fp32, name="z1")
                    nc.scalar.activation(z1[:], pz1[:], ACT.Relu, bias=bc1[:])
                    pz2 = mpp.tile([64, 1], fp32, tag="pz", name="pz2")
                    nc.tensor.matmul(pz2[:], wc2[:], z1[:], start=True, stop=True)
                    z2 = mp.tile([64, 1], fp32, name="z2")
                    nc.scalar.activation(z2[:], pz2[:], ACT.Relu, bias=bc2[:])
                    pz3 = mpp.tile([8, 1], fp32, tag="pz", name="pz3")
                    nc.tensor.matmul(pz3[:], wc3[:], z2[:64, :], start=True, stop=True)
                    zo = mp.tile([8, 1], fp32, name="zo")
                    nc.scalar.activation(zo[:], pz3[:], ACT.Identity, bias=bc3[:])
                    nc.sync.dma_start(out_d[:], zo[:])
                plp_cm.__exit__(None, None, None)

            for _rep in range(repeat):
                run_body(_rep)

    nc.compile()
    return nc


# --------------------------------------------------------------------------
# entry point
# --------------------------------------------------------------------------

# interleave: new feature j <-> original head-major index (j%4)*64 + j//4
_PERM = np.array([(j % HEADS) * FH + j // HEADS for j in range(HID)])


def kernel(**inputs):
    x = np.asarray(inputs["x"], dtype=np.float32)
    ei = np.asarray(inputs["edge_index"], dtype=np.int64)
    sched = _preprocess(x, ei)
    nc = _build(sched)

    W = {k: np.asarray(v, dtype=np.float32) for k, v in inputs.items()
         if k not in ("x", "edge_index")}

    def pack_k(w, nslab):   # [K, M] -> [128, nslab*M] (row-slab packed)
        K, M = w.shape
        out = np.zeros((128, nslab * M), np.float32)
        for s in range(nslab):
            r0 = s * 128
            r1 = min(K, r0 + 128)
            out[0:r1 - r0, s * M:(s + 1) * M] = w[r0:r1]
        return out

    # Wg with interleaved output cols + al_s/al_d projection columns
    wg_perm = W["Wg"][:, _PERM]                                   # [128, 256]
    wg_als = np.stack([W["Wg"][:, h * FH:(h + 1) * FH] @ W["a_src"][h]
                       for h in range(HEADS)], axis=1)            # [128, 4]
    wg_ald = np.stack([W["Wg"][:, h * FH:(h + 1) * FH] @ W["a_dst"][h]
                       for h in range(HEADS)], axis=1)            # [128, 4]
    wg_ext = np.concatenate([wg_perm, wg_als, wg_ald], axis=1)    # [128, 264]

    common = {
        "w1_d": pack_k(W["W1"], 1).astype(F16),
        "w2_d": pack_k(W["W2"], 2).astype(F16),
        "w3_d": pack_k(W["W3"], 2).astype(F16),
        "wg_d": wg_ext.astype(F16),
        "b1_d": W["b1"].reshape(1, -1).astype(F16),
        "b2_d": W["b2"].reshape(1, -1).astype(F16),
        "b3_d": W["b3"].reshape(1, -1).astype(F16),
        "bg_d": np.tile(W["bg"][_PERM].reshape(1, -1), (128, 1)).astype(F16),
        "wc1_d": pack_k(W["Wc1"][_PERM, :], 2).astype(np.float32),
        "wc2_d": pack_k(W["Wc2"], 1)[:, :64].astype(np.float32),
        "wc3_d": pack_k(W["Wc3"], 1)[:64, :8].astype(np.float32),
        "bc1_d": W["bc1"].reshape(-1, 1).astype(np.float32),
        "bc2_d": W["bc2"].reshape(-1, 1).astype(np.float32),
        "bc3_d": W["bc3"].reshape(-1, 1).astype(np.float32),
        "rowmask_d": (np.arange(128) < LASTB).astype(np.float32).reshape(128, 1),
    }

    NCHs = sched["NCH"]
    dinv_full = sched["dinv_full"]
    in_maps = []
    for c in range(NCORES):
        g1 = (x[c * NPC:(c + 1) * NPC]
              * dinv_full[c * NPC:(c + 1) * NPC, None]).astype(F16)
        in_maps.append(dict(
            common,
            g1in=np.ascontiguousarray(g1),
            idxs_d=sched["idxs"][c],
            dstrel_d=np.ascontiguousarray(
                sched["dstrel"][c].reshape(NCHs, 128).T).astype(np.float32),
            dinvd_d=np.ascontiguousarray(
                sched["dinvd"][c].reshape(NCHs, 128).T).astype(np.float32),
            dinv_d=sched["dinv"][c],
        ))

    res = run_bass_kernel_spmd(nc, in_maps, core_ids=list(range(NCORES)))
    global LAST_RESULT
    LAST_RESULT = res
    return res.results[0]["out_d"].reshape(1, OUT).astype(np.float32)


LAST_RESULT = None


# revision 30
# speedup vs baseline: 1.0006x; 1.0006x over previous
"""Trainium2 Bass kernel for the EnhancedGNNDetector (3x GCN + GAT + pool + MLP).

Strategy (8 NeuronCores, SPMD single program):
  - Nodes sharded contiguously: core c owns dsts [c*6250, (c+1)*6250).
  - Edges (with self-loops) sorted by dst, partitioned by dst owner, then by
    (4-block group, src-half piece).  Within each (group, piece) run the edges
    are chunked into 128s regardless of block boundaries; a chunk crossing a
    block boundary issues one extra S-build + matmul.  Chunk counts per run
    are padded to the cross-core max so one static program serves all cores.
  - Aggregation: per chunk a one-hot S matrix (tensor_scalar is_equal against
    an iota row, dst-relative values as the per-partition scalar) turns the
    segment-sum into PE matmuls accumulating in PSUM.  For the GCN layers the
    dst-side 1/sqrt(deg) is folded into S via op1=mult.
  - Tables: host pre-scales x by dinv (g1), per layer the dinv-src-scaled
    features are written fp16 to a local DRAM table and AllGathered.
    src index is int16, so tables are gathered in two halves.
  - GAT: table rows are [hg fp16 interleaved (f,h) 256 | al_s f32 (8 slots) |
    pad] = 384 fp16 slots.  al_s/al_d come from extra host-packed Wg columns
    (Wg @ a_src per head).  al_d per edge is computed on device via
    S^T (PE transpose) matmul against SBUF-resident (al_d - c).  The softmax
    shift c is global per head; exp(leaky(u)-c) = max(exp(u-c),
    exp(0.2u)*exp(-0.8c)) so only Exp activations are needed.  Exp weights
    are written into message cols [256:260]; the aggregation matmul carries
    260 cols so the softmax denominator falls out of the same PSUM.
"""

import numpy as np
import concourse.bacc as bacc
import concourse.bass as bass
import concourse.mybir as mybir
import concourse.tile as tile
from concourse.bass_utils import run_bass_kernel_spmd

F16 = np.float16
N = 50000
E = 800000
NCORES = 8
NPC = N // NCORES            # 6250 nodes per core
NB = (NPC + 127) // 128      # 49 dst blocks per core
LASTB = NPC - 128 * (NB - 1)  # 106 rows in last block
HALF = 32768                 # int16 gather split
GBLK = 4                     # blocks per group
NG = (NB + GBLK - 1) // GBLK  # 13 groups (last has 1 block)
D_IN = 128
HID = 256
H3D = 128
HEADS = 4
FH = 64
GSLOT = 384                  # GAT table row slots (fp16)
SUB = 17                     # max chunks per gather sub-span
DM = HID + 4                 # GAT message cols incl exp weights
OUT = 8
NEG = 0.2

fp16 = mybir.dt.float16
fp32 = mybir.dt.float32
i16 = mybir.dt.int16
ALU = mybir.AluOpType
ACT = mybir.ActivationFunctionType


# --------------------------------------------------------------------------
# host-side schedule + per-core streams
# --------------------------------------------------------------------------

def _preprocess(x, edge_index):
    src = np.concatenate([edge_index[0], np.arange(N, dtype=np.int64)])
    dst = np.concatenate([edge_index[1], np.arange(N, dtype=np.int64)])
    deg = np.bincount(dst, minlength=N).astype(np.float32)
    dinv = np.where(deg > 0, 1.0 / np.sqrt(deg), 0.0).astype(np.float32)

    order = np.argsort(dst, kind="stable")
    s_src, s_dst = src[order], dst[order]

    core = s_dst // NPC
    blk = (s_dst % NPC) // 128
    grp = blk // GBLK
    piece = (s_src >= HALF).astype(np.int64)

    key = (core * NG + grp) * 2 + piece
    korder = np.argsort(key, kind="stable")   # stable: keeps dst order inside
    k_src, k_dst, k_key = s_src[korder], s_dst[korder], key[korder]
    bounds = np.searchsorted(k_key, np.arange(NCORES * NG * 2 + 1))
    cnt = (bounds[1:] - bounds[:-1]).reshape(NCORES, NG, 2)
    run_ch = (-(-cnt // 128)).max(axis=0)     # [NG, 2] cross-core chunk counts

    # canonical chunk order: for g, for piece, run chunks
    spans = []           # (start_chunk, n_chunks, piece, group)
    pos = 0
    for g in range(NG):
        for p in range(2):
            n = int(run_ch[g, p])
            if n:
                spans.append((pos, n, p, g))
                pos += n
    NCH = pos

    # per-core streams + cross-core chunk block range
    def wrap(stream):
        return np.ascontiguousarray(np.tile(stream.reshape(-1, 16).T.copy(), (8, 1)))

    blo = np.full(NCH, 1 << 30, np.int64)
    bhi = np.full(NCH, -1, np.int64)
    idxs_all, dstrel_all, dinvd_all = [], [], []
    for c in range(NCORES):
        idx_stream = np.zeros(NCH * 128, np.int16)
        rel_stream = np.full(NCH * 128, -1.0, np.float32)
        dvd_stream = np.zeros(NCH * 128, np.float32)
        for (start, n_ch, p, g) in spans:
            k = (c * NG + g) * 2 + p
            e0, e1 = bounds[k], bounds[k + 1]
            n = e1 - e0
            cap = n_ch * 128
            assert n <= cap
            es, ed = k_src[e0:e1], k_dst[e0:e1]
            q = start * 128
            idx_stream[q:q + n] = (es - (HALF if p else 0)).astype(np.int16)
            rel = ed - c * NPC - g * GBLK * 128       # rel to group base
            rel_stream[q:q + n] = rel.astype(np.float32)
            dvd_stream[q:q + n] = dinv[ed]
            # chunk block range (within group), cross-core union
            eblk = rel // 128
            for kk in range(n_ch):
                s0, s1 = kk * 128, min(n, kk * 128 + 128)
                if s0 >= s1:
                    continue
                b0 = g * GBLK + int(eblk[s0])
                b1 = g * GBLK + int(eblk[s1 - 1])
                gi = start + kk
                blo[gi] = min(blo[gi], b0)
                bhi[gi] = max(bhi[gi], b1)
        idxs_all.append(wrap(idx_stream))
        dstrel_all.append(rel_stream)
        dinvd_all.append(dvd_stream)

    # entries: (chunk, block, iota_off) per chunk, static across cores
    entries = []         # per chunk: list of (block, off)
    for (start, n_ch, p, g) in spans:
        for kk in range(n_ch):
            gi = start + kk
            ent = []
            if bhi[gi] >= 0:
                for b in range(int(blo[gi]), int(bhi[gi]) + 1):
                    ent.append((b, b - g * GBLK))
            else:
                ent.append((g * GBLK, 0))   # fully-pad chunk: harmless target
            entries.append(ent)
    total_mm = np.zeros(NB, np.int64)
    for ent in entries:
        for (b, off) in ent:
            total_mm[b] += 1

    dinv_blocks = []
    for c in range(NCORES):
        dv = np.ones(NB * 128, np.float32)
        dv[:NPC] = dinv[c * NPC:(c + 1) * NPC]
        dinv_blocks.append(np.ascontiguousarray(dv.reshape(NB, 128).T))  # [128, NB]

    groups = []
    for g in range(NG):
        gspans = [s for s in spans if s[3] == g]
        blocks = list(range(g * GBLK, min((g + 1) * GBLK, NB)))
        groups.append((blocks, gspans))

    return {
        "NCH": NCH, "spans": spans, "groups": groups, "entries": entries,
        "total_mm": total_mm,
        "idxs": idxs_all, "dstrel": dstrel_all, "dinvd": dinvd_all,
        "dinv": dinv_blocks, "dinv_full": dinv,
    }


# --------------------------------------------------------------------------
# device program
# --------------------------------------------------------------------------

def _build(sched, repeat=1, no_cc=False):
    NCH = sched["NCH"]
    groups = sched["groups"]
    entries = sched["entries"]
    total_mm = sched["total_mm"]

    nc = bacc.Bacc("TRN2", target_bir_lowering=False, debug=False,
                   num_devices=NCORES, num_swdge_queues=4)

    # ---------------- external tensors ----------------
    g1in = nc.dram_tensor("g1in", [NPC, D_IN], fp16, kind="ExternalInput")
    idxs_d = nc.dram_tensor("idxs_d", [128, NCH * 8], i16, kind="ExternalInput")
    dstrel_d = nc.dram_tensor("dstrel_d", [128, NCH], fp32, kind="ExternalInput")
    dinvd_d = nc.dram_tensor("dinvd_d", [128, NCH], fp32, kind="ExternalInput")
    dinv_d = nc.dram_tensor("dinv_d", [128, NB], fp32, kind="ExternalInput")
    w1_d = nc.dram_tensor("w1_d", [128, HID], fp16, kind="ExternalInput")
    w2_d = nc.dram_tensor("w2_d", [128, 2 * HID], fp16, kind="ExternalInput")
    w3_d = nc.dram_tensor("w3_d", [128, 2 * H3D], fp16, kind="ExternalInput")
    wg_d = nc.dram_tensor("wg_d", [128, HID + 8], fp16, kind="ExternalInput")
    b1_d = nc.dram_tensor("b1_d", [1, HID], fp16, kind="ExternalInput")
    b2_d = nc.dram_tensor("b2_d", [1, HID], fp16, kind="ExternalInput")
    b3_d = nc.dram_tensor("b3_d", [1, H3D], fp16, kind="ExternalInput")
    bg_d = nc.dram_tensor("bg_d", [128, HID], fp16, kind="ExternalInput")
    wc1_d = nc.dram_tensor("wc1_d", [128, 2 * 128], fp32, kind="ExternalInput")
    wc2_d = nc.dram_tensor("wc2_d", [128, 64], fp32, kind="ExternalInput")
    wc3_d = nc.dram_tensor("wc3_d", [64, 8], fp32, kind="ExternalInput")
    bc1_d = nc.dram_tensor("bc1_d", [128, 1], fp32, kind="ExternalInput")
    bc2_d = nc.dram_tensor("bc2_d", [64, 1], fp32, kind="ExternalInput")
    bc3_d = nc.dram_tensor("bc3_d", [8, 1], fp32, kind="ExternalInput")
    rowmask_d = nc.dram_tensor("rowmask_d", [128, 1], fp32, kind="ExternalInput")
    out_d = nc.dram_tensor("out_d", [8, 1], fp32, kind="ExternalOutput")

    # internal DRAM tables
    g1loc = nc.dram_tensor("g1loc", [NPC, D_IN], fp16)
    g1full = nc.dram_tensor("g1full", [N, D_IN], fp16, addr_space="Shared")
    g2loc = nc.dram_tensor("g2loc", [NPC, HID], fp16)
    g2full = nc.dram_tensor("g2full", [N, HID], fp16, addr_space="Shared")
    g3loc = nc.dram_tensor("g3loc", [NPC, H3D], fp16)
    g3full = nc.dram_tensor("g3full", [N, H3D], fp16, addr_space="Shared")
    gtloc = nc.dram_tensor("gtloc", [NPC, GSLOT], fp16)
    gtfull = nc.dram_tensor("gtfull", [N, GSLOT], fp16, addr_space="Shared")
    arin = nc.dram_tensor("arin", [128, 2], fp32)
    arout = nc.dram_tensor("arout", [128, 2], fp32, addr_space="Shared")

    RG = [list(range(NCORES))]

    with tile.TileContext(nc) as tc:
        import contextlib
        es = contextlib.ExitStack()
        with es:
            pers = es.enter_context(tc.tile_pool(name="pers", bufs=1))
            # ---------- persistent SBUF ----------
            idxs = pers.tile([128, NCH * 8], i16)
            nc.sync.dma_start(idxs[:], idxs_d[:])
            dstrel = pers.tile([128, NCH], fp32)
            nc.sync.dma_start(dstrel[:], dstrel_d[:])
            dinvd = pers.tile([128, NCH], fp32)
            nc.sync.dma_start(dinvd[:], dinvd_d[:])
            dinv = pers.tile([128, NB], fp32)
            nc.sync.dma_start(dinv[:], dinv_d[:])

            w1 = pers.tile([128, HID], fp16); nc.sync.dma_start(w1[:], w1_d[:])
            w2 = pers.tile([128, 2 * HID], fp16); nc.sync.dma_start(w2[:], w2_d[:])
            w3 = pers.tile([128, 2 * H3D], fp16); nc.sync.dma_start(w3[:], w3_d[:])
            wg = pers.tile([128, HID + 8], fp16); nc.sync.dma_start(wg[:], wg_d[:])
            b1r = pers.tile([1, HID], fp16); nc.sync.dma_start(b1r[:], b1_d[:])
            b2r = pers.tile([1, HID], fp16); nc.sync.dma_start(b2r[:], b2_d[:])
            b3r = pers.tile([1, H3D], fp16); nc.sync.dma_start(b3r[:], b3_d[:])
            bgr = pers.tile([128, HID], fp16); nc.sync.dma_start(bgr[:], bg_d[:])
            wc1 = pers.tile([128, 2 * 128], fp32); nc.sync.dma_start(wc1[:], wc1_d[:])
            wc2 = pers.tile([128, 64], fp32); nc.sync.dma_start(wc2[:], wc2_d[:])
            wc3 = pers.tile([64, 8], fp32); nc.sync.dma_start(wc3[:], wc3_d[:])
            bc1 = pers.tile([128, 1], fp32); nc.sync.dma_start(bc1[:], bc1_d[:])
            bc2 = pers.tile([64, 1], fp32); nc.sync.dma_start(bc2[:], bc2_d[:])
            bc3 = pers.tile([8, 1], fp32); nc.sync.dma_start(bc3[:], bc3_d[:])
            rowmask = pers.tile([128, 1], fp32); nc.sync.dma_start(rowmask[:], rowmask_d[:])

            # iota rows for S builds (values off*128 .. off*128+127), offs 0..3
            iotas = []
            for off in range(GBLK):
                it_i = pers.tile([128, 128], i16, name=f"it_i{off}")
                nc.gpsimd.iota(it_i[:], pattern=[[1, 128]], base=off * 128,
                               channel_multiplier=0)
                it_f = pers.tile([128, 128], fp16, name=f"it_f{off}")
                nc.vector.tensor_copy(it_f[:], it_i[:])
                iotas.append(it_f)
            iop_i = pers.tile([128, 1], i16)
            nc.gpsimd.iota(iop_i[:], pattern=[[1, 1]], base=0, channel_multiplier=1)
            iop_f = pers.tile([128, 1], fp16)
            nc.vector.tensor_copy(iop_f[:], iop_i[:])
            ident = pers.tile([128, 128], fp16)
            nc.vector.tensor_tensor(
                ident[:], iop_f[:].broadcast_to([128, 128]), iotas[0][:],
                op=ALU.is_equal)
            ones_r = pers.tile([1, 128], fp16)
            nc.vector.memset(ones_r[:], 1.0)
            ones_c = pers.tile([128, 1], fp16)
            nc.vector.memset(ones_c[:], 1.0)

            als_all = pers.tile([128, NB, HEADS], fp32)
            ald_all = pers.tile([128, NB, HEADS], fp32)
            ald_c = pers.tile([128, NB, HEADS], fp16)
            crep = pers.tile([128, HEADS], fp32)
            krep = pers.tile([128, HEADS], fp16)

            def rows(b):
                return LASTB if b == NB - 1 else 128

            # ---------- helpers ----------
            def transpose_to_sbuf(pool, psum_pool, src16, nslab, tag):
                out = pool.tile([128, nslab, 128], fp16, tag=tag, name=f"tT_{tag}")
                pt = psum_pool.tile([128, nslab, 128], fp16, tag="tr", name="pt_tr", bufs=1)
                for s in range(nslab):
                    nc.tensor.transpose(pt[:, s, :], src16[:, s * 128:(s + 1) * 128], ident[:])
                nc.scalar.copy(out[:], pt[:])
                return out

            qctr = [0]

            def next_q():
                qctr[0] += 1
                return qctr[0] % 4

            def gather_into(m_tile, table, start_chunk, n_chunks, elem):
                nc.gpsimd.dma_gather(
                    m_tile[:, 0:n_chunks, :], table,
                    idxs[:, start_chunk * 8:(start_chunk + n_chunks) * 8],
                    num_idxs=n_chunks * 128, num_idxs_reg=n_chunks * 128,
                    elem_size=elem, single_packet=False, queue_num=next_q())

            def maybe_cc(kind, op, replica_groups, ins, outs):
                if no_cc:
                    nrow = ins[0].shape[0]
                    if nrow >= 1024:
                        h = nrow // 2
                        nc.sync.dma_start(outs[0].tensor[0:h], ins[0][0:h])
                        nc.sync.dma_start(outs[0].tensor[h:nrow], ins[0][h:nrow])
                    else:
                        nc.sync.dma_start(outs[0].tensor[0:nrow], ins[0])
                else:
                    nc.gpsimd.collective_compute(kind, op, replica_groups=replica_groups,
                                                 ins=ins, outs=outs)

            def build_S(pool, start, n_ch, tag, weighted):
                """One S tile per span; one is_equal per (chunk, block-entry)."""
                nent = sum(len(entries[start + kk]) for kk in range(n_ch))
                S = pool.tile([128, nent, 128], fp16, tag=tag, name=f"S_{tag}", bufs=4)
                ei = 0
                emap = []    # (chunk kk, block, entry index)
                for kk in range(n_ch):
                    for (b, off) in entries[start + kk]:
                        if weighted:
                            nc.vector.tensor_scalar(
                                S[:, ei, :], iotas[off][:],
                                dstrel[:, start + kk:start + kk + 1],
                                dinvd[:, start + kk:start + kk + 1],
                                op0=ALU.is_equal, op1=ALU.mult)
                        else:
                            nc.vector.tensor_scalar(
                                S[:, ei, :], iotas[off][:],
                                dstrel[:, start + kk:start + kk + 1], None,
                                op0=ALU.is_equal)
                        emap.append((kk, b, ei))
                        ei += 1
                return S, emap

            def run_body(rep):
                nc.sync.dma_start(g1loc[:], g1in[:])
                maybe_cc("AllGather", ALU.bypass, RG, [g1loc[:]], [g1full[:]])

                # ================= GCN layer runner =================
                h1_pool = tc.tile_pool(name=f"h1pool_{rep}", bufs=1)
                h1_ctx = h1_pool.__enter__()
                h1_all = h1_ctx.tile([128, NB, HID], fp16)

                def gcn_layer(lname, table_full, D, evict_fn, stop_in_evict=False,
                              weighted_S=False):
                    with (tc.tile_pool(name=f"{lname}_sb_{rep}", bufs=2) as lp,
                          tc.tile_pool(name=f"{lname}_ps_{rep}", bufs=5, space="PSUM") as pp,
                          tc.tile_pool(name=f"{lname}_wps_{rep}", bufs=2, space="PSUM") as wp):
                        tab_lo = table_full[0:HALF, :]
                        tab_hi = table_full[HALF:N, :]
                        for (blocks, gspans) in groups:
                            paggs = {}
                            first = {}
                            done = {b: 0 for b in blocks}
                            for b in blocks:
                                paggs[b] = pp.tile([128, D], fp32, tag="agg", name="pagg")
                                first[b] = True
                            for (start0, n_ch0, p, g) in gspans:
                                tab = tab_hi if p else tab_lo
                                for s0 in range(0, n_ch0, SUB):
                                    start = start0 + s0
                                    n_ch = min(SUB, n_ch0 - s0)
                                    m = lp.tile([128, n_ch, D], fp16, tag="m", name="m", bufs=6)
                                    gather_into(m, tab, start, n_ch, D)
                                    S, emap = build_S(lp, start, n_ch, "s", weighted=weighted_S)
                                    for (kk, b, ei) in emap:
                                        done[b] += 1
                                        stop = (done[b] == total_mm[b]) and not stop_in_evict
                                        nc.tensor.matmul(
                                            paggs[b][:], S[:, ei, :], m[:, kk, :],
                                            start=first[b], stop=stop)
                                        first[b] = False
                            for b in blocks:
                                evict_fn(b, paggs[b], lp, wp)

                # ---------- layer 1 ----------
                def evict1(b, pagg, lp, wp):
                    r = rows(b)
                    a1s = lp.tile([128, D_IN], fp16, tag="ev1", name="a1s")
                    nc.scalar.activation(a1s[:], pagg[:], ACT.Copy, scale=dinv[:, b:b + 1])
                    a1T = transpose_to_sbuf(lp, wp, a1s, 1, "ev1T")
                    ph = wp.tile([128, HID], fp32, tag="wout", name="ph1")
                    nc.tensor.matmul(ph[:], a1T[:, 0, :], w1[:], start=True, stop=False)
                    nc.tensor.matmul(ph[:], ones_r[:], b1r[:], start=False, stop=True)
                    h1t = h1_all[:, b, :]
                    nc.scalar.activation(h1t, ph[:], ACT.Relu)
                    g2t = lp.tile([128, HID], fp16, tag="ev1g", name="g2t")
                    nc.scalar.activation(g2t[:], h1t, ACT.Copy, scale=dinv[:, b:b + 1])
                    nc.sync.dma_start(g2loc[b * 128:b * 128 + r, :], g2t[:r, :])

                gcn_layer("L1", g1full, D_IN, evict1)
                maybe_cc("AllGather", ALU.bypass, RG, [g2loc[:]], [g2full[:]])

                # ---------- layer 2 (+ residual + L3 transform) ----------
                def evict2(b, pagg, lp, wp):
                    r = rows(b)
                    a2s = lp.tile([128, HID], fp16, tag="ev2", name="a2s")
                    nc.scalar.activation(a2s[:], pagg[:], ACT.Copy, scale=dinv[:, b:b + 1])
                    a2T = transpose_to_sbuf(lp, wp, a2s, 2, "ev2T")
                    ph = wp.tile([128, HID], fp32, tag="wout", name="ph2")
                    nc.tensor.matmul(ph[:], a2T[:, 0, :], w2[:, 0:HID], start=True, stop=False)
                    nc.tensor.matmul(ph[:], a2T[:, 1, :], w2[:, HID:2 * HID], start=False, stop=False)
                    nc.tensor.matmul(ph[:], ones_r[:], b2r[:], start=False, stop=True)
                    r2 = lp.tile([128, HID], fp16, tag="ev2r", name="r2")
                    nc.scalar.activation(r2[:], ph[:], ACT.Relu)
                    h2t16 = lp.tile([128, HID], fp16, tag="ev2h6", name="h2t16")
                    nc.vector.tensor_tensor(h2t16[:], r2[:], h1_all[:, b, :], op=ALU.add)
                    h2T = transpose_to_sbuf(lp, wp, h2t16, 2, "ev2hT")
                    pt3 = wp.tile([128, H3D], fp32, tag="wout", name="pt3")
                    nc.tensor.matmul(pt3[:], h2T[:, 0, :], w3[:, 0:H3D], start=True, stop=False)
                    nc.tensor.matmul(pt3[:], h2T[:, 1, :], w3[:, H3D:2 * H3D], start=False, stop=True)
                    g3t = lp.tile([128, H3D], fp16, tag="ev2g", name="g3t")
                    nc.scalar.activation(g3t[:], pt3[:], ACT.Copy, scale=dinv[:, b:b + 1])
                    nc.sync.dma_start(g3loc[b * 128:b * 128 + r, :], g3t[:r, :])

                gcn_layer("L2", g2full, HID, evict2)
                h1_pool.__exit__(None, None, None)
                maybe_cc("AllGather", ALU.bypass, RG, [g3loc[:]], [g3full[:]])

                # ---------- layer 3 aggregation + GAT prep ----------
                def evict3(b, pagg, lp, wp):
                    r = rows(b)
                    # bias add closes the aggregation psum
                    nc.tensor.matmul(pagg[:], ones_r[:], b3r[:], start=False, stop=True)
                    h3t16 = lp.tile([128, H3D], fp16, tag="ev3h", name="h3t16")
                    nc.scalar.activation(h3t16[:], pagg[:], ACT.Relu)
                    h3T = transpose_to_sbuf(lp, wp, h3t16, 1, "ev3T")
                    phg = wp.tile([128, HID + 8], fp32, tag="wout", name="phg")
                    nc.tensor.matmul(phg[:], h3T[:, 0, :], wg[:], start=True, stop=True)
                    nc.scalar.copy(als_all[:, b, :], phg[:, HID:HID + 4])
                    nc.scalar.copy(ald_all[:, b, :], phg[:, HID + 4:HID + 8])
                    tabt = lp.tile([128, GSLOT], fp16, tag="ev3tab", name="tabt")
                    nc.scalar.copy(tabt[:, 0:HID], phg[:, 0:HID])
                    nc.scalar.copy(tabt[:, HID:HID + 4], phg[:, HID:HID + 4])
                    nc.sync.dma_start(gtloc[b * 128:b * 128 + r, :], tabt[:r, :])

                gcn_layer("L3", g3full, H3D, evict3, stop_in_evict=True, weighted_S=True)

                # shift constants c[h] = max(ub, 0.2*ub), ub = max al_s + max al_d
                cps = contextlib.ExitStack()
                cp = cps.enter_context(tc.tile_pool(name=f"cp_{rep}", bufs=1))
                cpp = cps.enter_context(tc.tile_pool(name=f"cpp_{rep}", bufs=1, space="PSUM"))
                m1 = cp.tile([128, HEADS], fp32)
                nc.vector.tensor_reduce(
                    m1[:], als_all[:].rearrange("p b h -> p h b"),
                    axis=mybir.AxisListType.X, op=ALU.max)
                m2 = cp.tile([128, HEADS], fp32)
                nc.vector.tensor_reduce(
                    m2[:], ald_all[:].rearrange("p b h -> p h b"),
                    axis=mybir.AxisListType.X, op=ALU.max)
                m1_16 = cp.tile([128, HEADS], fp16)
                nc.vector.tensor_copy(m1_16[:], m1[:])
                m2_16 = cp.tile([128, HEADS], fp16)
                nc.vector.tensor_copy(m2_16[:], m2[:])
                pmt1 = cpp.tile([HEADS, 128], fp16, tag="pmt1", name="pmt1")
                nc.tensor.transpose(pmt1[:], m1_16[:], ident[:])
                pmt2 = cpp.tile([HEADS, 128], fp16, tag="pmt2", name="pmt2")
                nc.tensor.transpose(pmt2[:], m2_16[:], ident[:])
                mt = cp.tile([HEADS, 2 * 128], fp32)
                nc.scalar.copy(mt[:, 0:128], pmt1[:])
                nc.scalar.copy(mt[:, 128:256], pmt2[:])
                ms = cp.tile([HEADS, 2], fp32)
                nc.vector.tensor_reduce(
                    ms[:], mt[:].rearrange("p (a j) -> p a j", a=2),
                    axis=mybir.AxisListType.X, op=ALU.max)
                ub = cp.tile([HEADS, 1], fp32)
                nc.vector.tensor_tensor(ub[:], ms[:, 0:1], ms[:, 1:2], op=ALU.add)
                ub2 = cp.tile([HEADS, 1], fp32)
                nc.vector.tensor_scalar(ub2[:], ub[:], 0.2, None, op0=ALU.mult)
                cc = cp.tile([HEADS, 1], fp32)
                nc.vector.tensor_tensor(cc[:], ub[:], ub2[:], op=ALU.max)
                cc16 = cp.tile([HEADS, 1], fp16)
                nc.vector.tensor_copy(cc16[:], cc[:])
                pcr = cpp.tile([1, HEADS], fp16)
                nc.tensor.transpose(pcr[:], cc16[:HEADS, :], ident[0:HEADS, 0:HEADS])
                pcr_sb = cp.tile([1, HEADS], fp16)
                nc.scalar.copy(pcr_sb[:], pcr[:])
                pcrep = cpp.tile([128, HEADS], fp32)
                nc.tensor.matmul(pcrep[:], ones_r[:], pcr_sb[:], start=True, stop=True)
                nc.scalar.copy(crep[:], pcrep[:])
                # krep = exp(-0.8 c); ald_c = al_d - c (fp16)
                nc.scalar.activation(krep[:], crep[:], ACT.Exp, scale=-0.8)
                nc.vector.tensor_tensor(
                    ald_c[:], ald_all[:],
                    crep[:].unsqueeze(1).broadcast_to([128, NB, HEADS]),
                    op=ALU.subtract)
                cps.close()

                maybe_cc("AllGather", ALU.bypass, RG, [gtloc[:]], [gtfull[:]])

                # ================= GAT layer =================
                plp_cm = tc.tile_pool(name=f"pool_ps_{rep}", bufs=1, space="PSUM")
                plp = plp_cm.__enter__()
                ppool = plp.tile([128, 2], fp32, tag="pp", name="ppool")
                ppool0 = ppool[:, 0:1]
                ppool1 = ppool[:, 1:2]
                with (tc.tile_pool(name=f"gat_sb_{rep}", bufs=2) as gp,
                      tc.tile_pool(name=f"gat_ps_{rep}", bufs=4, space="PSUM") as gpp,
                      tc.tile_pool(name=f"gat_tps_{rep}", bufs=2, space="PSUM") as tpp):
                    tab_lo = gtfull[0:HALF, :]
                    tab_hi = gtfull[HALF:N, :]
                    first = {b: True for b in range(NB)}
                    done = {b: 0 for b in range(NB)}
                    paggs = {}

                    def gat_span(start, n_ch, tab, pgsb, densb, bidx):
                        m = gp.tile([128, n_ch, GSLOT], fp16, tag="gm", name="gm", bufs=5)
                        gather_into(m, tab, start, n_ch, GSLOT)
                        S, emap = build_S(gp, start, n_ch, "gs", weighted=False)
                        nent = len(emap)
                        # S^T for al_d-per-edge, batched 8 per psum bank
                        ST = gp.tile([128, nent, 128], fp16, tag="gst", name="gst", bufs=4)
                        for i0 in range(0, nent, 8):
                            nb8 = min(8, nent - i0)
                            pst = tpp.tile([128, 8, 128], fp16, tag="st", name="pst", bufs=2)
                            for j in range(nb8):
                                nc.tensor.transpose(pst[:, j, :], S[:, i0 + j, :], ident[:])
                            nc.scalar.copy(ST[:, i0:i0 + nb8, :], pst[:, 0:nb8, :])
                        # u = al_s + (al_d - c) via S^T matmul
                        ups = tpp.tile([128, n_ch, HEADS], fp32, tag="u", name="ups", bufs=1)
                        for kk in range(n_ch):
                            nc.tensor.matmul(
                                ups[:, kk, :], ident[:], m[:, kk, HID:HID + 4],
                                start=True, stop=False)
                        for (kk, b, ei) in emap:
                            last = (ei == nent - 1) or emap[ei + 1][0] != kk
                            nc.tensor.matmul(
                                ups[:, kk, :], ST[:, ei, :], ald_c[:, b, :],
                                start=False, stop=last)
                        e1 = gp.tile([128, n_ch, HEADS], fp16, tag="ge1", name="ge1", bufs=3)
                        nc.scalar.activation(
                            e1[:].rearrange("p a b -> p (a b)"),
                            ups[:].rearrange("p a b -> p (a b)"), ACT.Exp)
                        e2 = gp.tile([128, n_ch, HEADS], fp16, tag="ge2", name="ge2", bufs=3)
                        nc.scalar.activation(
                            e2[:].rearrange("p a b -> p (a b)"),
                            ups[:].rearrange("p a b -> p (a b)"), ACT.Exp, scale=NEG)
                        nc.vector.tensor_tensor(
                            e2[:], e2[:],
                            krep[:].unsqueeze(1).broadcast_to([128, n_ch, HEADS]),
                            op=ALU.mult)
                        nc.vector.tensor_tensor(
                            m[:, :, HID:HID + 4], e1[:], e2[:], op=ALU.max)
                        nc.vector.tensor_tensor(
                            m[:, :, 0:HID].rearrange("p k (f h) -> p k f h", h=HEADS),
                            m[:, :, 0:HID].rearrange("p k (f h) -> p k f h", h=HEADS),
                            m[:, :, HID:HID + 4].unsqueeze(2).broadcast_to(
                                [128, n_ch, FH, HEADS]),
                            op=ALU.mult)
                        for (kk, b, ei) in emap:
                            done[b] += 1
                            stop = (done[b] == total_mm[b])
                            nc.tensor.matmul(
                                paggs[b][:], S[:, ei, :], m[:, kk, 0:DM],
                                start=first[b], stop=stop)
                            first[b] = False
                            if stop:
                                i = bidx[b]
                                pg = paggs.pop(b)
                                nc.scalar.copy(pgsb[:, i, :], pg[:, 0:HID])
                                nc.scalar.copy(densb[:, i, :], pg[:, HID:HID + 4])

                    def gat_evict_group(blocks, pgsb, densb):
                        nb = len(blocks)
                        nc.vector.tensor_scalar(
                            densb[:, 0:nb, :], densb[:, 0:nb, :], 1e-30, None,
                            op0=ALU.max)
                        rden = gp.tile([128, GBLK, HEADS], fp32, tag="grden", name="grden")
                        nc.vector.reciprocal(rden[:, 0:nb, :], densb[:, 0:nb, :])
                        t2 = gp.tile([128, GBLK, HID], fp16, tag="gt2", name="gt2")
                        nc.vector.tensor_tensor(
                            t2[:, 0:nb, :].rearrange("p b (f h) -> p b f h", h=HEADS),
                            pgsb[:, 0:nb, :].rearrange("p b (f h) -> p b f h", h=HEADS),
                            rden[:, 0:nb, :].unsqueeze(2).broadcast_to(
                                [128, nb, FH, HEADS]),
                            op=ALU.mult)
                        nc.vector.tensor_tensor(
                            t2[:, 0:nb, :], t2[:, 0:nb, :],
                            bgr[:].unsqueeze(1).broadcast_to([128, nb, HID]),
                            op=ALU.add)
                        hatt = gp.tile([128, GBLK, HID], fp16, tag="ghat", name="ghat")
                        nc.scalar.activation(
                            hatt[:, 0:nb, :].rearrange("p b d -> p (b d)"),
                            t2[:, 0:nb, :].rearrange("p b d -> p (b d)"), ACT.Relu)
                        for i, b in enumerate(blocks):
                            if rows(b) < 128:
                                nc.vector.tensor_scalar(
                                    hatt[:, i, :], hatt[:, i, :], rowmask[:], None,
                                    op0=ALU.mult)
                            nc.tensor.matmul(ppool0, hatt[:, i, 0:128], ones_c[:],
                                             start=(b == 0), stop=(b == NB - 1))
                            nc.tensor.matmul(ppool1, hatt[:, i, 128:256], ones_c[:],
                                             start=(b == 0), stop=(b == NB - 1))

                    for (blocks, gspans) in groups:
                        for b in blocks:
                            paggs[b] = gpp.tile([128, DM], fp32, tag="gagg", name="gagg")
                        pgsb = gp.tile([128, GBLK, HID], fp16, tag="gpg", name="gpg")
                        densb = gp.tile([128, GBLK, HEADS], fp32, tag="gden", name="gden")
                        bidx = {b: i for i, b in enumerate(blocks)}
                        for (start0, n_ch0, p, g) in gspans:
                            for s0 in range(0, n_ch0, SUB):
                                gat_span(start0 + s0, min(SUB, n_ch0 - s0),
                                         tab_hi if p else tab_lo, pgsb, densb, bidx)
                        gat_evict_group(blocks, pgsb, densb)

                # ---------- pooling + AllReduce + MLP ----------
                with (tc.tile_pool(name=f"mlp_sb_{rep}", bufs=1) as mp,
                      tc.tile_pool(name=f"mlp_ps_{rep}", bufs=1, space="PSUM") as mpp):
                    pool_sb = mp.tile([128, 2], fp32, name="pool_sb")
                    nc.scalar.copy(pool_sb[:], ppool[:])
                    nc.sync.dma_start(arin[:], pool_sb[:])
                    maybe_cc("AllReduce", ALU.add, RG, [arin[:]], [arout[:]])
                    pooled = mp.tile([128, 2], fp32, name="pooled")
                    nc.sync.dma_start(pooled[:], arout[:])
                    nc.vector.tensor_scalar(pooled[:], pooled[:], 1.0 / N, None, op0=ALU.mult)
                    pz1 = mpp.tile([128, 1], fp32, tag="pz", name="pz1")
                    nc.tensor.matmul(pz1[:], wc1[:, 0:128], pooled[:, 0:1], start=True, stop=False)
                    nc.tensor.matmul(pz1[:], wc1[:, 128:256], pooled[:, 1:2], start=False, stop=True)
                    z1 = mp.tile([128, 1], fp32, name="z1")
                    nc.scalar.activation(z1[:], pz1[:], ACT.Relu, bias=bc1[:])
                    pz2 = mpp.tile([64, 1], fp32, tag="pz", name="pz2")
                    nc.tensor.matmul(pz2[:], wc2[:], z1[:], start=True, stop=True)
                    z2 = mp.tile([64, 1], fp32, name="z2")
                    nc.scalar.activation(z2[:], pz2[:], ACT.Relu, bias=bc2[:])
                    pz3 = mpp.tile([8, 1], fp32, tag="pz", name="pz3")
                    nc.tensor.matmul(pz3[:], wc3[:], z2[:64, :], start=True, stop=True)
                    zo = mp.tile([8, 1], fp32, name="zo")
                    nc.scalar.activation(zo[:], pz3[:], ACT.Identity, bias=bc3[:])
                    nc.sync.dma_start(out_d[:], zo[:])
                plp_cm.__exit__(None, None, None)

            for _rep in range(repeat):
                run_body(_rep)

    nc.compile()
    return nc


# --------------------------------------------------------------------------
# entry point
# --------------------------------------------------------------------------

# interleave: new feature j <-> original head-major index (j%4)*64 + j//4
_PERM = np.array([(j % HEADS) * FH + j // HEADS for j in range(HID)])


def kernel(**inputs):
    x = np.asarray(inputs["x"], dtype=np.float32)
    ei = np.asarray(inputs["edge_index"], dtype=np.int64)
    sched = _preprocess(x, ei)
    nc = _build(sched)

    W = {k: np.asarray(v, dtype=np.float32) for k, v in inputs.items()
         if k not in ("x", "edge_index")}

    def pack_k(w, nslab):   # [K, M] -> [128, nslab*M] (row-slab packed)
        K, M = w.shape
        out = np.zeros((128, nslab * M), np.float32)
        for s in range(nslab):
            r0 = s * 128
            r1 = min(K, r0 + 128)
            out[0:r1 - r0, s * M:(s + 1) * M] = w[r0:r1]
        return out

    # Wg with interleaved output cols + al_s/al_d projection columns
    wg_perm = W["Wg"][:, _PERM]                                   # [128, 256]
    wg_als = np.stack([W["Wg"][:, h * FH:(h + 1) * FH] @ W["a_src"][h]
                       for h in range(HEADS)], axis=1)            # [128, 4]
    wg_ald = np.stack([W["Wg"][:, h * FH:(h + 1) * FH] @ W["a_dst"][h]
                       for h in range(HEADS)], axis=1)            # [128, 4]
    wg_ext = np.concatenate([wg_perm, wg_als, wg_ald], axis=1)    # [128, 264]

    common = {
        "w1_d": pack_k(W["W1"], 1).astype(F16),
        "w2_d": pack_k(W["W2"], 2).astype(F16),
        "w3_d": pack_k(W["W3"], 2).astype(F16),
        "wg_d": wg_ext.astype(F16),
        "b1_d": W["b1"].reshape(1, -1).astype(F16),
        "b2_d": W["b2"].reshape(1, -1).astype(F16),
        "b3_d": W["b3"].reshape(1, -1).astype(F16),
        "bg_d": np.tile(W["bg"][_PERM].reshape(1, -1), (128, 1)).astype(F16),
        "wc1_d": pack_k(W["Wc1"][_PERM, :], 2).astype(np.float32),
        "wc2_d": pack_k(W["Wc2"], 1)[:, :64].astype(np.float32),
        "wc3_d": pack_k(W["Wc3"], 1)[:64, :8].astype(np.float32),
        "bc1_d": W["bc1"].reshape(-1, 1).astype(np.float32),
        "bc2_d": W["bc2"].reshape(-1, 1).astype(np.float32),
        "bc3_d": W["bc3"].reshape(-1, 1).astype(np.float32),
        "rowmask_d": (np.arange(128) < LASTB).astype(np.float32).reshape(128, 1),
    }

    NCHs = sched["NCH"]
    dinv_full = sched["dinv_full"]
    in_maps = []
    for c in range(NCORES):
        g1 = (x[c * NPC:(c + 1) * NPC]
              * dinv_full[c * NPC:(c + 1) * NPC, None]).astype(F16)
        in_maps.append(dict(
            common,
            g1in=np.ascontiguousarray(g1),
            idxs_d=sched["idxs"][c],
            dstrel_d=np.ascontiguousarray(
                sched["dstrel"][c].reshape(NCHs, 128).T).astype(np.float32),
            dinvd_d=np.ascontiguousarray(
                sched["dinvd"][c].reshape(NCHs, 128).T).astype(np.float32),
            dinv_d=sched["dinv"][c],
        ))

    res = run_bass_kernel_spmd(nc, in_maps, core_ids=list(range(NCORES)))
    global LAST_RESULT
    LAST_RESULT = res
    return res.results[0]["out_d"].reshape(1, OUT).astype(np.float32)


LAST_RESULT = None
